# revision 33
# baseline (speedup 1.0000x reference)
"""Trainium2 Bass kernel: causal MHSA, last-position output (fp32, N-small matmuls).

The reference returns only out[:, -1, :]; with the causal mask the last query
row attends to everything, so per batch element the whole MHSA collapses to
tiny GEMVs (q_row and M = Wk-contracted-with-q fold on the host, removing the
Wq/Wk transfers and the x@Wk / x@Wv matmuls entirely).  Per-core device cost:
stream x (2MB) + Wv/Wo (1MB) from HBM, ~90 matmuls.  Sharding: pure data
parallel over batch, core b <- batch b, no collectives.

The two big matmuls are emitted in transposed form so the streamed (free) dimension is 8 instead of 512/256 —
fp32 matmul cost scales with the free dim (4 cyc/row), while the 128-col
weight loads ride the separate LDW port:

    scores^T tiles [s,8] = xT_chunk.T @ M_chunk      (lhsT = xT, N=8)
    -> exp lands directly in the [s-part, h] layout the attention matmul
       needs, so the w-transpose stage disappears;
    attn^T chunks [f,8]  = x_chunk.T @ w_tile        (lhsT = x,  N=8)
    -> lands directly in the [f-part, h] layout the Wv matmul needs, so the
       attn_x transpose stage disappears.
    softmax sums via ones[128,1].T @ w_tiles accumulation (partition-dim sum).

Everything is fp32 end-to-end (no fp32r): HW rel err ~1.5e-6.
"""

import numpy as np
from contextlib import ExitStack

import concourse.bass as bass
import concourse.tile as tile
from concourse import bacc, mybir
from concourse.bass_utils import run_bass_kernel_spmd
from concourse.masks import make_identity

B, S, F, PROJ, H, D = 8, 2048, 256, 512, 8, 64
NT = S // 128        # 16 s-tiles
FC = F // 128        # 2 f-chunks
SG = 4               # s-tiles per pipeline group
NG = NT // SG        # 4 groups
f32 = mybir.dt.float32
EXP = mybir.ActivationFunctionType.Exp

_cache = {}


def _build():
    nc = bacc.Bacc("TRN2", target_bir_lowering=False, debug=False, num_devices=B)
    x = nc.dram_tensor("x", [S, F], f32, kind="ExternalInput").ap()
    M = nc.dram_tensor("M", [F, H], f32, kind="ExternalInput").ap()
    Wv = nc.dram_tensor("Wv", [F, PROJ], f32, kind="ExternalInput").ap()
    Wo = nc.dram_tensor("Wo", [PROJ, F], f32, kind="ExternalInput").ap()
    bo = nc.dram_tensor("bo", [FC, 128], f32, kind="ExternalInput").ap()
    # 0/1 selectors for the block-diag recip pattern: bd = A.T @ (B * recip)
    Abd = nc.dram_tensor("Abd", [H, 128], f32, kind="ExternalInput").ap()
    Bbd = nc.dram_tensor("Bbd", [H, 4], f32, kind="ExternalInput").ap()
    out = nc.dram_tensor("out", [F], f32, kind="ExternalOutput").ap()

    with tile.TileContext(nc) as tc, ExitStack() as ctx:
        P = ctx.enter_context(tc.tile_pool(name="persist", bufs=1))
        xtp = ctx.enter_context(tc.tile_pool(name="xtp", bufs=3, space="PSUM"))
        sct = ctx.enter_context(tc.tile_pool(name="sct", bufs=1, space="PSUM"))
        pers = ctx.enter_context(tc.tile_pool(name="pers", bufs=1, space="PSUM"))
        axp = ctx.enter_context(tc.tile_pool(name="axp", bufs=2, space="PSUM"))
        tailp = ctx.enter_context(tc.tile_pool(name="tailp", bufs=1, space="PSUM"))

        ident = P.tile([128, 128], f32)
        ones_col = P.tile([128, 1], f32)
        x_sb = P.tile([128, NT, F], f32)
        xT_sb = P.tile([128, FC, S], f32)
        m_sb = P.tile([128, FC, H], f32)
        wv_sb = P.tile([128, FC, PROJ], f32)
        wo_sb = P.tile([128, 4, F], f32)
        bo_sb = P.tile([1, FC, 128], f32)
        wt_sb = P.tile([128, NT * H], f32)
        srecip = P.tile([H, 1], f32)
        axT_sb = P.tile([128, FC * H], f32)
        abd_sb = P.tile([H, 128], f32)
        bbd_sb = P.tile([H, 4], f32)
        bw_sb = P.tile([H, 4], f32)
        bd_sb = P.tile([128, 4], f32)
        ac_sb = P.tile([128, 4], f32)
        o_sb = P.tile([128, FC], f32)
        dummy = P.tile([1, 1], f32)

        # trigger the ACT Exp table load early, overlapped with DMA
        nc.vector.memset(dummy[:], 0.0)
        nc.scalar.activation(out=dummy[:], in_=dummy[:], func=EXP)
        nc.vector.memset(ones_col[:], 1.0)

        make_identity(nc, ident[:])

        # ---- DMAs: x group 0 in halves (earlier compute start), rest of x,
        #      tiny M between, tail weights
        xr = x.rearrange("(t p) f -> p t f", p=128)
        nc.sync.dma_start(out=x_sb[:, 0:2, :], in_=xr[:, 0:2, :])
        nc.sync.dma_start(out=x_sb[:, 2:SG, :], in_=xr[:, 2:SG, :])
        nc.sync.dma_start(out=x_sb[:, SG : 2 * SG, :], in_=xr[:, SG : 2 * SG, :])
        nc.sync.dma_start(out=m_sb[:], in_=M.rearrange("(c p) h -> p c h", p=128))
        for g in range(2, NG):
            nc.sync.dma_start(
                out=x_sb[:, g * SG : (g + 1) * SG, :], in_=xr[:, g * SG : (g + 1) * SG, :]
            )
        nc.sync.dma_start(out=wv_sb[:], in_=Wv.rearrange("(c p) n -> p c n", p=128))
        nc.sync.dma_start(out=wo_sb[:], in_=Wo.rearrange("(c p) n -> p c n", p=128))
        nc.sync.dma_start(out=bo_sb[0:1, :, :], in_=bo[:])
        nc.sync.dma_start(out=abd_sb[:], in_=Abd[:])
        nc.sync.dma_start(out=bbd_sb[:], in_=Bbd[:])

        # ---- PE warm-up: open the HAM clock gate while DMA streams
        warm_ps = xtp.tile([128, SG * 128], f32, tag="xt")
        for j in range(8):
            nc.tensor.transpose(
                warm_ps[:, (j % SG) * 128 : (j % SG + 1) * 128], ident[:], ident[:]
            )

        # persistent PSUM accumulators
        sums_ps = pers.tile([H, 1], f32, tag="sums")
        axc_ps = [
            pers.tile([128, H], f32, tag=f"axc{c}", name=f"axc_ps{c}") for c in range(FC)
        ]

        # ---- pipelined per 512-row group
        for g in range(NG):
            lo = g * SG * 128
            for c in range(FC):
                xt_ps = xtp.tile([128, SG * 128], f32, tag="xt")
                for j in range(SG):
                    nc.tensor.transpose(
                        xt_ps[:, j * 128 : (j + 1) * 128],
                        x_sb[:, g * SG + j, c * 128 : (c + 1) * 128],
                        ident[:],
                    )
                nc.vector.tensor_copy(xT_sb[:, c, lo : lo + SG * 128], xt_ps[:])
            # scores^T tiles [128, 8] per s-tile, N=8 matmuls
            sct_ps = sct.tile([128, SG * H], f32, tag="sc")
            for j in range(SG):
                for c in range(FC):
                    nc.tensor.matmul(
                        sct_ps[:, j * H : (j + 1) * H],
                        xT_sb[:, c, lo + j * 128 : lo + (j + 1) * 128],
                        m_sb[:, c, :],
                        start=(c == 0),
                        stop=(c == FC - 1),
                    )
            # exp straight into the [s-part, h] layout attention needs
            nc.scalar.activation(
                out=wt_sb[:, g * SG * H : (g + 1) * SG * H],
                in_=sct_ps[:],
                func=EXP,
                scale=0.125,
            )
            # attn^T chunks [f-part, h] and softmax sums column, accumulated
            for j in range(SG):
                t_idx = g * SG + j
                nc.tensor.matmul(
                    sums_ps[:],
                    wt_sb[:, t_idx * H : (t_idx + 1) * H],
                    ones_col[:],
                    start=(t_idx == 0),
                    stop=(t_idx == NT - 1),
                    skip_group_check=True,
                )
                for c in range(FC):
                    nc.tensor.matmul(
                        axc_ps[c][:],
                        x_sb[:, t_idx, c * 128 : (c + 1) * 128],
                        wt_sb[:, t_idx * H : (t_idx + 1) * H],
                        start=(t_idx == 0),
                        stop=(t_idx == NT - 1),
                        skip_group_check=True,
                    )

        # ---- softmax denominator: reciprocal straight off the PSUM column
        nc.vector.reciprocal(srecip[:], sums_ps[:])

        # block-diag recip pattern bd[j, c] = recip[2c + (j>=64)] via one matmul
        nc.vector.tensor_scalar_mul(bw_sb[:], bbd_sb[:], srecip[:])
        bd_ps = tailp.tile([128, 4], f32, tag="tail")
        nc.tensor.matmul(bd_ps[:], abd_sb[:], bw_sb[:], start=True, stop=True)
        nc.vector.tensor_copy(bd_sb[:], bd_ps[:])

        # ---- attn^T to SBUF (already in [f-part, h] layout for the Wv matmul)
        for c in range(FC):
            nc.vector.tensor_copy(axT_sb[:, c * H : (c + 1) * H], axc_ps[c][:])

        # ---- attn_full^T blocks [p-part, h]: afT = Wv_block.T @ axT, N=8
        afT_ps = xtp.tile([128, 4 * H], f32, tag="xt")
        for pc in range(4):
            for c in range(FC):
                nc.tensor.matmul(
                    afT_ps[:, pc * H : (pc + 1) * H],
                    wv_sb[:, c, pc * 128 : (pc + 1) * 128],
                    axT_sb[:, c * H : (c + 1) * H],
                    start=(c == 0),
                    stop=(c == FC - 1),
                )
        # afT[j, 8pc+h] = attn_f[h, 128pc+j]; extract col 10c + (j>=64) per chunk,
        # normalizing by the block-diag recip pattern on the way out
        top = afT_ps[0:64, 0:1]
        bot = afT_ps[64:128, 1:2]
        nc.vector.tensor_mul(
            ac_sb[0:64, 0:4],
            bass.AP(tensor=top.tensor, offset=top.offset, ap=[top.ap[0], [10, 4]]),
            bd_sb[0:64, 0:4],
        )
        nc.vector.tensor_mul(
            ac_sb[64:128, 0:4],
            bass.AP(tensor=bot.tensor, offset=bot.offset, ap=[bot.ap[0], [10, 4]]),
            bd_sb[64:128, 0:4],
        )

        # ---- out[256] = attn_col.T @ Wo + bo  (column layout [128, 2]);
        #      bias enters as a rank-1 accumulation, result DMAs out of PSUM
        o_ps = tailp.tile([128, FC], f32, tag="tail")
        for mc in range(FC):
            for c in range(4):
                nc.tensor.matmul(
                    o_ps[:, mc : mc + 1],
                    wo_sb[:, c, mc * 128 : (mc + 1) * 128],
                    ac_sb[:, c : c + 1],
                    start=(c == 0),
                    stop=False,
                    skip_group_check=True,
                )
            nc.tensor.matmul(
                o_ps[:, mc : mc + 1],
                bo_sb[0:1, mc, :],
                ones_col[0:1, 0:1],
                start=False,
                stop=True,
                skip_group_check=True,
            )
        nc.vector.tensor_copy(o_sb[:], o_ps[:])
        nc.sync.dma_start(out=out.rearrange("(c p) -> p c", p=128), in_=o_sb[:])

    nc.compile()
    return nc


def get_nc():
    if "nc" not in _cache:
        _cache["nc"] = _build()
    return _cache["nc"]


def host_prep(inputs: dict) -> list[dict]:
    """Per-core input maps: x slice + host-folded M + shared Wv/Wo/bo."""
    xs = np.ascontiguousarray(np.asarray(inputs["x"], dtype=np.float32))
    Wq = np.asarray(inputs["Wq"], dtype=np.float32)
    Wk = np.asarray(inputs["Wk"], dtype=np.float32)
    shared = {
        k: np.ascontiguousarray(np.asarray(inputs[k], dtype=np.float32))
        for k in ("Wv", "Wo")
    }
    shared["bo"] = np.ascontiguousarray(
        np.asarray(inputs["bo"], dtype=np.float32).reshape(FC, 128)
    )
    j = np.arange(128)
    h = np.arange(H)
    shared["Abd"] = np.ascontiguousarray(
        ((h[:, None] % 2) == (j[None, :] >= 64)).astype(np.float32)
    )
    shared["Bbd"] = np.ascontiguousarray(
        ((h[:, None] // 2) == np.arange(4)[None, :]).astype(np.float32)
    )
    in_maps = []
    for b in range(B):
        q_row = xs[b, -1] @ Wq                                   # [512]
        Mb = (Wk * q_row[None, :]).reshape(F, H, D).sum(-1)      # [256, 8]
        in_maps.append({"x": xs[b], "M": np.ascontiguousarray(Mb), **shared})
    return in_maps


def run_hw(inputs: dict) -> np.ndarray:
    nc = get_nc()
    res = run_bass_kernel_spmd(nc, host_prep(inputs), list(range(B)))
    return np.stack([res.results[b]["out"] for b in range(B)])


def kernel(**inputs) -> np.ndarray:
    return run_hw(inputs)
